# revision 3
# baseline (speedup 1.0000x reference)
"""Trainium2 Bass kernel for nn_Attention_41472204210940.

Reference computation (per batch b):
    q = x @ Wq; k, v = split(x @ Wkv); multi-head attention (H=8, DH=64);
    out = attn_out @ Wout + bout.

Sharding over 8 NeuronCores: core c handles batch b = c//2 and head group
g = c%2 (heads 4g..4g+4, i.e. inner-dim columns 256g..256g+256 of
Wq/Wk/Wv column-parallel and rows 256g..256g+256 of Wout row-parallel).
Each core emits a partial [2048, 512] output; the host unshard sums the
two partials per batch and adds bout.

Per-core program (bf16 matmul operands, fp32 PSUM accumulation):
  - QT/KT = W.T @ xT in [inner, N] layout; V natural [N, inner] plus a
    ones column per head so P @ V_aug also yields softmax denominators.
  - per (head-pair, query-block, key-chunk): ST[j, i] = K^T Q, then
    P = exp(SCALE * ST + mask_bias[j]).  The exp runs on the Scalar
    (ACT) engine for most key chunks, but a fixed subset per block is
    computed on the otherwise-idle Pool engine via a Schraudolph-style
    bit trick: i16 = rne(ST * (SCALE*128*log2e) + (16256 - C + mask))
    reinterpreted as bf16 IS exp (|rel err| ~2% sawtooth, unbiased at
    C=7.3; softmax denominators absorb the common scale).  This breaks
    the ACT bottleneck (exp on ACT alone costs ~139us vs the PE's
    ~139us of matmul streaming -- offloading ~25% restores PE pacing).
  - OT[d, i] += V_aug.T @ P accumulated over key chunks in PSUM; row DH
    holds denominators; epilogue normalizes into AOT (reciprocal +
    gpsimd partition-broadcast + multiply; odd head via bounce tile +
    gpsimd SBUF DMA).
  - out[t] = sum_pair AOT_pair[:, t].T @ Wout_pair, K=128 chains.

Scheduling: the projection / output-projection chains are injected into
the attention blocks' PE slack so neither PE nor ACT ever idles long:
lead-in emits only K0 (all), Q0 t0 and V0..V5, so the first exp starts
~9us in; V6..15 + Q0t1 ride block 0, Q0t2/Q1t0/K1t0 block 1, etc.; the
final projection rides the second pass with lag 1 (only the last 4
chunks drain after the loop).
"""

import numpy as np

B, N, D = 4, 2048, 512
H_TOTAL, DH = 8, 64
HEADS = 4            # heads per core
INNER = HEADS * DH   # per-core inner width (256)
N_CORES = 8
SCALE = DH ** -0.5


def build_program(n=N, d=D, heads=HEADS, dh=DH,
                  pool_exp_jcs=(2, 6, 10, 14), trick_c=7.3,
                  final_lag=1, p_bufs=10, lead_v=6,
                  attn_prio=True, warmup_mms=12):
    """Build + compile the per-core Bass program (SPMD; all cores run the
    identical program on different data)."""
    import contextlib
    import concourse.bacc as bacc
    import concourse.mybir as mybir
    from concourse import tile

    f32 = mybir.dt.float32
    bf = mybir.dt.bfloat16
    i16 = mybir.dt.int16
    u8 = mybir.dt.uint8
    AF = mybir.ActivationFunctionType
    Alu = mybir.AluOpType

    inner = heads * dh
    KC = d // 128          # k-chunks of the projection contraction dim
    IC = inner // 128      # 128-row chunks of QT/KT == head pairs
    NJ = n // 128          # key chunks
    NI = n // 512          # query blocks
    VW = dh + 1            # V columns per head incl. the ones column

    LOG2E = 1.4426950408889634
    trick_a = SCALE * 128.0 * LOG2E          # logit -> bf16-bit scale
    s2_unmask = 16256.0 - trick_c            # 127<<7 minus sawtooth offset
    MASK_DROP = 42000.0                      # masked -> i16 ~ -25744 (tiny neg bf16)
    pool_set = frozenset(pool_exp_jcs)

    assert dh == 64 and inner % 128 == 0 and n % 512 == 0 and d % 128 == 0
    assert NI == 4 and NJ == 16 and IC == 2 and KC == 4

    nc = bacc.Bacc("TRN2", target_bir_lowering=False, debug=False)

    xt_d = nc.dram_tensor("xt", [d, n], bf, kind="ExternalInput")
    wq_d = nc.dram_tensor("wq", [d, inner], bf, kind="ExternalInput")
    wk_d = nc.dram_tensor("wk", [d, inner], bf, kind="ExternalInput")
    wv_d = nc.dram_tensor("wv", [d, inner], bf, kind="ExternalInput")
    wo_d = nc.dram_tensor("wo", [inner, d], bf, kind="ExternalInput")
    mask_d = nc.dram_tensor("mask", [n], u8, kind="ExternalInput")
    out_d = nc.dram_tensor("out", [n, d], f32, kind="ExternalOutput")

    with tile.TileContext(nc) as tc:
        with (
            nc.allow_low_precision(reason="bf16 matmul operand prep"),
            tc.tile_pool(name="const", bufs=1) as cpool,
            tc.tile_pool(name="pwork", bufs=p_bufs) as ppool,
            tc.tile_pool(name="small", bufs=2) as spool,
            tc.tile_pool(name="outsb", bufs=3) as opool,
            tc.tile_pool(name="mm", bufs=2, space="PSUM") as mmpool,
            tc.tile_pool(name="ot", bufs=2, space="PSUM") as otpool,
        ):
            # block 0's OT accumulator from the mm pool, allocated first:
            # its PV can start as soon as V[0] exists (block 0 runs with
            # single-slot STs until its epilogue frees this tile)
            ot0 = mmpool.tile([VW, 1024], f32, tag="mm", name="ot0")

            # ---- input loads (bf16 from the host shard step); fused
            # column-band tiles, few large strided DMAs ----
            xTa = cpool.tile([128, KC * n], bf, name="xTa")
            wqa = cpool.tile([128, KC * inner], bf, name="wqa")
            wka = cpool.tile([128, KC * inner], bf, name="wka")
            wva = cpool.tile([128, KC * inner], bf, name="wva")
            wo = [cpool.tile([128, d], bf, name=f"wo{i}") for i in range(IC)]

            def xT(k):
                return xTa[:, n * k:n * (k + 1)]

            def wslice(wa, k):
                return wa[:, inner * k:inner * (k + 1)]

            masku8 = cpool.tile([128, NJ], u8, name="masku8")
            nc.sync.dma_start(
                out=masku8[:], in_=mask_d[:].rearrange("(c p) -> p c", p=128)
            )
            xt_r = xt_d[:].rearrange("(k p) c -> p k c", p=128)
            for t in range(NI):
                ts = slice(512 * t, 512 * (t + 1))
                nc.sync.dma_start(
                    out=xTa[:].rearrange("p (k c) -> p k c", c=n)[:, :, ts],
                    in_=xt_r[:, :, ts],
                )
                if t == 0:
                    # wk first: the K projection chains lead
                    for wa, wd in ((wka, wk_d), (wqa, wq_d)):
                        nc.scalar.dma_start(
                            out=wa[:].rearrange("p (k c) -> p k c", c=inner),
                            in_=wd[:].rearrange("(k p) c -> p k c", p=128),
                        )
                if t == min(1, NI - 1):
                    nc.sync.dma_start(
                        out=wva[:].rearrange("p (k c) -> p k c", c=inner),
                        in_=wv_d[:].rearrange("(k p) c -> p k c", p=128),
                    )
            for i in range(IC):
                nc.sync.dma_start(out=wo[i][:], in_=wo_d[128 * i:128 * (i + 1), :])

            # PE warmup: trip the HAM clock gate to 2.4GHz during DMA wait
            if warmup_mms:
                wup = cpool.tile([128, 512], bf, name="wup")
                nc.vector.memset(wup[:], 0.0)
                wps = mmpool.tile([128, 512], f32, tag="mm", name="wps")
                for i in range(warmup_mms):
                    nc.tensor.matmul(
                        wps[:], wup[:, 0:128], wup[:],
                        start=(i == 0), stop=(i == warmup_mms - 1),
                    )

            # ACT-exp bias: 0 if kept, -1e30 if masked
            maskb = cpool.tile([128, NJ], f32, name="maskb")
            nc.vector.tensor_scalar(
                maskb[:], masku8[:], -1.0, 1e30, Alu.add, Alu.mult
            )
            # Pool-exp bit-trick bias: 16256-C if kept, -25744-C if masked
            s2 = cpool.tile([128, NJ], f32, name="s2")
            nc.vector.tensor_scalar(
                s2[:], masku8[:], MASK_DROP, s2_unmask - MASK_DROP,
                Alu.mult, Alu.add
            )

            onesh_f = cpool.tile([128, heads], f32, name="onesh_f")
            nc.vector.memset(onesh_f[:], 1.0)

            QT = [cpool.tile([128, n], bf, name=f"QT{m}") for m in range(IC)]
            KT = [cpool.tile([128, n], bf, name=f"KT{m}") for m in range(IC)]
            V = [cpool.tile([128, heads * VW], bf, name=f"V{j}") for j in range(NJ)]
            AOT = [cpool.tile([128, n], bf, name=f"AOT{m}") for m in range(IC)]

            # ---- projection chains; psum from the ot pool so injected
            # chains never steal the ST double-buffer from the mm pool ----
            def qk_proj_one(m, chain):
                W, OUT = ((wqa, QT), (wka, KT))[chain % 2]
                t = chain // 2
                ts = slice(512 * t, 512 * (t + 1))
                ps = otpool.tile([128, 512], f32, tag="ot", name="psqk")
                for k in range(KC):
                    nc.tensor.matmul(
                        ps[:],
                        wslice(W, k)[:, 128 * m:128 * (m + 1)],
                        xT(k)[:, ts],
                        start=(k == 0),
                        stop=(k == KC - 1),
                    )
                nc.vector.tensor_copy(OUT[m][:, ts], ps[:])

            def v_proj(j):
                ps = otpool.tile([128, inner], f32, tag="ot", name="psv")
                for k in range(KC):
                    nc.tensor.matmul(
                        ps[:],
                        xT(k)[:, 128 * j:128 * (j + 1)],
                        wslice(wva, k),
                        start=(k == 0),
                        stop=(k == KC - 1),
                    )
                vv = V[j][:].rearrange("p (h e) -> p h e", e=VW)
                nc.vector.tensor_copy(
                    vv[:, :, 0:dh], ps[:].rearrange("p (h v) -> p h v", v=dh)
                )
                nc.vector.tensor_copy(
                    vv[:, :, dh:VW],
                    onesh_f[:].rearrange("p (h o) -> p h o", o=1),
                )

            def final_proj(t, from_mm=False):
                if from_mm:
                    ps = mmpool.tile([128, d], f32, tag="mm", name="psf")
                else:
                    ps = otpool.tile([128, d], f32, tag="ot", name="psf")
                for ic in range(IC):
                    nc.tensor.matmul(
                        ps[:],
                        AOT[ic][:, 128 * t:128 * (t + 1)],
                        wo[ic][:],
                        start=(ic == 0),
                        stop=(ic == IC - 1),
                    )
                ob = opool.tile([128, d], f32, tag="ob", name="ob")
                if t % 2 == 1:
                    nc.scalar.activation(ob[:], ps[:], AF.Copy)
                else:
                    nc.vector.tensor_copy(ob[:], ps[:])
                nc.sync.dma_start(out=out_d[128 * t:128 * (t + 1), :], in_=ob[:])

            # ---- lead-in: K0 fully, Q0 t0, V0..lead_v-1 ----
            for t in range(NI):
                qk_proj_one(0, 2 * t + 1)          # K0 chains
            qk_proj_one(0, 0)                      # Q0 t0
            for j in range(lead_v):
                v_proj(j)

            # ---- attention block: injected work runs between the exp
            # issue and the PV matmuls, hiding the exp latency ----
            def attn_block(ih, pr, injections, ot=None):
                isl = slice(512 * ih, 512 * (ih + 1))
                if ot is None:
                    ot = otpool.tile([VW, 1024], f32, tag="ot", name="ot")
                for jc in range(NJ):
                    jsl = slice(128 * jc, 128 * (jc + 1))
                    st = mmpool.tile([128, 1024], f32, tag="mm", name="st")
                    for hh in range(2):
                        rsl = slice(64 * hh, 64 * (hh + 1))
                        nc.tensor.matmul(
                            st[:, 512 * hh:512 * (hh + 1)],
                            KT[pr][rsl, jsl],
                            QT[pr][rsl, isl],
                            start=True,
                            stop=True,
                        )
                    if jc in pool_set:
                        # bit-trick exp on the DVE (Pool cannot read PSUM)
                        pi = ppool.tile([128, 1024], i16, tag="p", name="pi")
                        nc.vector.tensor_scalar(
                            pi[:], st[:], trick_a, s2[:, jc:jc + 1],
                            Alu.mult, Alu.add
                        )
                        p_ap = pi[:].bitcast(bf)
                    else:
                        p = ppool.tile([128, 1024], bf, tag="p", name="p")
                        nc.scalar.activation(
                            p[:], st[:], AF.Exp,
                            bias=maskb[:, jc:jc + 1], scale=SCALE,
                        )
                        p_ap = p[:]
                    for fn in injections.get(jc, ()):
                        fn()
                    for hh in range(2):
                        h = 2 * pr + hh
                        nc.tensor.matmul(
                            ot[:, 512 * hh:512 * (hh + 1)],
                            V[jc][:, VW * h:VW * (h + 1)],
                            p_ap[:, 512 * hh:512 * (hh + 1)],
                            start=(jc == 0),
                            stop=(jc == NJ - 1),
                        )
                # normalize: AOT rows = OT rows 0..dh-1 times 1/denom.
                for hh in range(2):
                    csl = slice(512 * hh, 512 * (hh + 1))
                    rc = spool.tile([1, 512], f32, tag="rc", name="rc")
                    nc.vector.reciprocal(rc[:], ot[dh:VW, csl])
                    rcb = spool.tile([dh, 512], f32, tag="rcb", name="rcb")
                    nc.gpsimd.partition_broadcast(rcb[:], rc[:])
                    if hh == 0:
                        nc.vector.tensor_mul(
                            AOT[pr][0:dh, isl], ot[0:dh, csl], rcb[:]
                        )
                    else:
                        tb = spool.tile([dh, 512], bf, tag="tb", name="tb")
                        nc.vector.tensor_mul(tb[:], ot[0:dh, csl], rcb[:])
                        nc.gpsimd.dma_start(out=AOT[pr][64:128, isl], in_=tb[:])

            def add_inj(inj, slot, fn):
                inj.setdefault(slot, []).append(fn)

            prio_ctx = tc.high_priority if attn_prio else contextlib.nullcontext

            # ---- pass 0 (head pair 0) ----
            # b0: V[lead_v..15] at slot j-2, Q0t1 late
            # b1: Q0t2, Q1t0, K1t0;  b2: Q0t3, K1t1;  b3: Q1t1, K1t2
            for ih in range(NI):
                inj = {}
                if ih == 0:
                    for j in range(lead_v, NJ):
                        add_inj(inj, j - 2, lambda j=j: v_proj(j))
                    add_inj(inj, 1, lambda: qk_proj_one(0, 2))       # Q0 t1
                elif ih == 1:
                    add_inj(inj, 0, lambda: qk_proj_one(0, 4))       # Q0 t2
                    add_inj(inj, 4, lambda: qk_proj_one(1, 0))       # Q1 t0
                    add_inj(inj, 8, lambda: qk_proj_one(1, 1))       # K1 t0
                    add_inj(inj, 12, lambda: qk_proj_one(1, 3))     # K1 t1
                elif ih == 2:
                    add_inj(inj, 2, lambda: qk_proj_one(0, 6))       # Q0 t3
                    add_inj(inj, 8, lambda: qk_proj_one(1, 5))       # K1 t2
                elif ih == 3:
                    add_inj(inj, 2, lambda: qk_proj_one(1, 2))       # Q1 t1
                    add_inj(inj, 8, lambda: qk_proj_one(1, 7))       # K1 t3
                with prio_ctx():
                    attn_block(ih, 0, inj, ot=ot0 if ih == 0 else None)

            # ---- pass 1 (head pair 1); finals ride with lag final_lag ----
            for ih in range(NI):
                inj = {}
                if ih == 0:
                    add_inj(inj, 2, lambda: qk_proj_one(1, 4))       # Q1 t2
                elif ih == 1:
                    add_inj(inj, 2, lambda: qk_proj_one(1, 6))       # Q1 t3
                if ih >= final_lag:
                    base = 4 * (ih - final_lag)
                    for q, slot in enumerate((8, 10, 12, 14)):
                        add_inj(inj, slot, lambda t=base + q: final_proj(t))
                with prio_ctx():
                    attn_block(ih, IC - 1, inj)

            # remaining output-projection chunks
            for t in range(4 * (NI - final_lag), 4 * NI):
                final_proj(t, from_mm=(t % 2 == 0))

    nc.compile()
    return nc


_PROGRAM = None


def _get_program():
    global _PROGRAM
    if _PROGRAM is None:
        _PROGRAM = build_program()
    return _PROGRAM


def make_in_maps(x, mask, Wq, Wkv, Wout):
    """Host-side shard: slice + lay out the full inputs for each core.
    Matmul operands ship as bf16 (the same round-to-nearest-even a device
    cast would apply before a bf16 matmul)."""
    import ml_dtypes

    bf16 = ml_dtypes.bfloat16
    in_maps = []
    for c in range(N_CORES):
        b, g = c // 2, c % 2
        cs = slice(INNER * g, INNER * (g + 1))
        vs = slice(D + INNER * g, D + INNER * (g + 1))
        in_maps.append({
            "xt": np.ascontiguousarray(x[b].T.astype(bf16)),
            "wq": np.ascontiguousarray(Wq[:, cs].astype(bf16)),
            "wk": np.ascontiguousarray(Wkv[:, cs].astype(bf16)),
            "wv": np.ascontiguousarray(Wkv[:, vs].astype(bf16)),
            "wo": np.ascontiguousarray(Wout[cs, :].astype(bf16)),
            "mask": np.ascontiguousarray(mask[b]).astype(np.uint8),
        })
    return in_maps


def combine_outputs(results, bout):
    """Host-side unshard: sum the two row-parallel partials per batch, add bias."""
    out = np.zeros((B, N, D), np.float32)
    for c in range(N_CORES):
        out[c // 2] += results[c]["out"]
    out += np.asarray(bout, np.float32)[None, None, :]
    return out


def kernel(**inputs):
    x = np.asarray(inputs["x"], np.float32)
    mask = np.asarray(inputs["mask"])
    Wq = np.asarray(inputs["Wq"], np.float32)
    Wkv = np.asarray(inputs["Wkv"], np.float32)
    Wout = np.asarray(inputs["Wout"], np.float32)
    bout = np.asarray(inputs["bout"], np.float32)

    from concourse.bass_utils import run_bass_kernel_spmd

    nc = _get_program()
    in_maps = make_in_maps(x, mask, Wq, Wkv, Wout)
    res = run_bass_kernel_spmd(nc, in_maps, list(range(N_CORES))).results
    return combine_outputs(res, bout)


if __name__ == "__main__":
    rng = np.random.default_rng(0)
    s = 1.0 / np.sqrt(D)
    demo = {
        "x": rng.standard_normal((B, N, D), np.float32),
        "mask": np.ones((B, N), bool),
        "Wq": rng.uniform(-s, s, (D, INNER * 2)).astype(np.float32),
        "Wkv": rng.uniform(-s, s, (D, INNER * 4)).astype(np.float32),
        "Wout": rng.uniform(-s, s, (INNER * 2, D)).astype(np.float32),
        "bout": rng.uniform(-s, s, D).astype(np.float32),
    }
    out = kernel(**demo)
    print("kernel output", out.shape, out.dtype, float(np.abs(out).max()))
